# revision 27
# baseline (speedup 1.0000x reference)
"""Trainium2 Bass kernel for the ergodicity loss (v3: power-moment basis).

Math: for x[T=512, B=16, N=32, d=2] in [0,1]^2 and modes (k0,k1) in {0..9}^2:
    basis = cos(pi*k0*x0) * cos(pi*k1*x1)                    (separable)
    coeffs[b, k0, k1] = sum_{t,n} basis / (T*N) / nf[k1]
    loss = mean((nw * (coeffs - cd))**2)

Device strategy (8 cores, data-parallel over T: 64 timesteps/core):
  - Instead of cos values, the device accumulates the MOMENT matrix
    Mo[b, m0, m1] = sum_{t,n} d0^m0 * d1^m1 with d = sin(pi*x - pi/2)
    = -cos(pi*x). The 10x10 Chebyshev transform cos(k*pi*x) = T_k(-d) =
    sum_m a[k,m] (-1)^m d^m is applied on the HOST (tiny [10,10] GEMMs),
    so the device only needs powers d^m -- pure multiplies, no subtracts:
      ACT:  d = Sin(pi*x - pi/2)   (arg in [-pi/2,pi/2], no range redux)
            d2 = Square(d)
      DVE:  d3 = d*d2; [d4,d6] = [d2,d3]^2; [d5,d7] = [d2,d3]*[d3,d4]
            d8 = d4^2          (tensor_tensor, bf16 2x mode)
      Pool: d9 = d4*d5         (otherwise-idle engine)
    bf16 end-to-end loss rel err ~6e-6 (verified vs numpy).
  - C layout [128, 64*80] bf16: 64 blocks (dd:2, f:16, g:2) of 80
    contiguous cols (pos:10, b:8) -- matmul operands are single-free-dim
    slices (HW requirement); elementwise ops use [p, 64, n*8] strided
    views (packed last dim). Position order pos->m = [0,1,2,3,4,6,5,7,8,9]
    makes every grouped operand/output a run of consecutive positions.
    pos0 = ones is memset ONCE per C buffer outside the loop (nothing in
    the loop writes it).
  - PE: per (f, batch-group g): lhsT = C[dd=0] block, rhs = C[dd=1] block
    -> PSUM[80, 80] accumulated over 16 f-chunks; off-diagonal batch
    blocks are garbage, ignored at gather.
  - Benchmark loop: For_i over reps/U with U=8 bodies unrolled, tile
    bufs=4 (C round-robin over 4 persistent buffers) so DMA latency,
    ACT, DVE, Pool and PE of successive iterations pipeline; the For_i
    all-engine barrier is amortized 1/U.
Host: sum 8 per-core [80,160] partials, extract diagonal batch blocks,
apply the A-transform + tiny [16,100] normalization + weighted MSE.
"""
import numpy as np

T, B, NA, D = 512, 16, 32, 2
KMAX = 10
NCORES = 8
TLOC = T // NCORES          # 64 timesteps per core
KN = KMAX * KMAX
# C slice order: position -> which power d^m it holds
POS2M = [0, 1, 2, 3, 4, 6, 5, 7, 8, 9]

_STATE = {}

CFG = {"unroll": 16, "bufs": 4, "cbufs": 4, "pool_ops": 2, "act_c2": True,
       "stagger": True, "psum_bufs": 2, "merge_so": False}


def _np_constants():
    """Replicates reference._constants() exactly in numpy (L = ones)."""
    L = np.ones(D, dtype=np.float32)
    grids = np.meshgrid(*[np.arange(KMAX) for _ in range(D)], indexing="ij")
    K = np.stack(grids, -1).reshape(-1, D).astype(np.float32)          # [100, 2]
    k_scaled = K * np.pi / L
    nf = np.where(K[:, -1] != 0, np.sqrt(L[-1] / 2.0), 1.0).astype(np.float32)
    nw = ((1.0 + (k_scaled ** 2).sum(-1)) ** (-(D + 1) / 2.0) * 100.0).astype(np.float32)
    safe_k = np.where(K != 0, k_scaled, 1.0)
    term = np.where(K != 0,
                    (np.exp(1j * k_scaled * L) - 1.0) / (1j * safe_k * L),
                    1.0 + 0j)
    cd = (term.prod(-1).real / nf).astype(np.float32)                  # [100]
    return nf, nw, cd


def _cheb_transform():
    """A'[k, m] with cos(k*pi*x) = sum_m A'[k,m] d^m, d = -cos(pi*x)."""
    A = np.zeros((KMAX, KMAX))
    A[0, 0] = 1.0
    A[1, 1] = 1.0
    for k in range(2, KMAX):
        A[k, 1:] += 2 * A[k - 1, :-1]
        A[k, :] -= A[k - 2, :]
    return A * ((-1.0) ** np.arange(KMAX))[None, :]


def _build(reps: int = 1, loop: bool = False, cfg: dict | None = None):
    import concourse.tile as tile
    from concourse import bacc, mybir

    cfg = {**CFG, **(cfg or {})}
    f32 = mybir.dt.float32
    bf16 = mybir.dt.bfloat16
    AF = mybir.ActivationFunctionType
    OP = mybir.AluOpType
    S = 512                       # elements per position slice
    PI = float(np.pi)

    nc = bacc.Bacc("TRN2", target_bir_lowering=False, debug=False)
    xx = nc.dram_tensor("xx", [128, 512], f32, kind="ExternalInput").ap()
    sout = nc.dram_tensor("sout", [80, 160], f32, kind="ExternalOutput").ap()

    with tile.TileContext(nc) as tc:
        with tc.tile_pool(name="cpool", bufs=1) as cpool, \
             tc.tile_pool(name="pool", bufs=cfg["bufs"]) as pool, \
             tc.tile_pool(name="ppool", bufs=cfg["psum_bufs"],
                          space="PSUM") as ppool:
            bias_sin = cpool.tile([128, 1], f32)
            nc.vector.memset(bias_sin[:], -PI / 2.0)

            # C buffers allocated once, used round-robin by the unrolled
            # bodies (unroll % cbufs == 0 keeps each body's buffer fixed).
            # pos0 = d^0 = ones, written once; the loop never writes pos0.
            NCB = cfg["cbufs"]
            C_list = [cpool.tile([128, 10 * S], bf16, name=f"Cbuf{j}")
                      for j in range(NCB)]
            for Ct in C_list:
                CVi = Ct[:].rearrange("p (c k b) -> p c k b",
                                      c=64, k=10, b=8)
                nc.vector.memset(CVi[:, :, 0, :], 1.0)

            # Engines execute their instruction streams IN ORDER, so a
            # stalled op (waiting on a cross-engine producer) blocks all
            # later-body work queued behind it. The loop is therefore
            # emitted SOFTWARE-PIPELINED: stage k of body u is emitted
            # alongside stage k-1 of body u+1, giving every cross-engine
            # dependency ~1 body of slack.
            def stages(u):
                C = C_list[u % NCB]
                CV = C[:].rearrange("p (c k b) -> p c k b", c=64, k=10, b=8)
                st = {"C": C, "CV": CV}

                def cs(p0):                 # single slice [p, 64, 8]
                    return CV[:, :, p0, :]

                def cr(p0, n):              # n consecutive positions, merged
                    v = CV[:, :, p0:p0 + n, :]
                    return v.rearrange("p c k b -> p c (k b)")

                def s0():                   # input DMA
                    XX = pool.tile([128, 512], f32, tag="XX")
                    st["XV"] = XX[:].rearrange("p (c b) -> p c b", c=64, b=8)
                    nc.sync.dma_start(XX[:], xx)

                def s1():                   # ACT: d -> pos1, d^2 -> pos2
                    nc.scalar.activation(cs(1), st["XV"], AF.Sin,
                                         bias=bias_sin[:], scale=PI)
                    if cfg["act_c2"]:
                        nc.scalar.activation(cs(2), cs(1), AF.Square,
                                             bias=0.0, scale=1.0)
                    else:
                        nc.vector.tensor_tensor(cs(2), cs(1), cs(1), OP.mult)

                def s2():                   # DVE/Pool powers
                    # pos: 3=d3 4=d4 5=d6 6=d5 7=d7 8=d8 9=d9
                    nc.vector.tensor_tensor(cs(3), cs(1), cs(2), OP.mult)
                    nc.vector.tensor_tensor(cr(4, 2), cr(2, 2), cr(2, 2),
                                            OP.mult)
                    nc.vector.tensor_tensor(cr(6, 2), cr(2, 2), cr(3, 2),
                                            OP.mult)
                    nc.vector.tensor_tensor(cs(8), cs(4), cs(4), OP.mult)
                    if cfg["pool_ops"] >= 1:
                        nc.gpsimd.tensor_tensor(cs(9), cs(4), cs(6), OP.mult)
                    else:
                        nc.vector.tensor_tensor(cs(9), cs(4), cs(6), OP.mult)

                def s3():                   # PE moment matmuls
                    C = st["C"]
                    if cfg["merge_so"]:
                        psw = ppool.tile([80, 160], f32, name="psw",
                                         tag="psw")
                        st["ps"] = psw
                        views = [psw[:, 0:80], psw[:, 80:160]]
                    else:
                        ps = [ppool.tile([80, 80], f32, name=f"ps{g}",
                                         tag=f"ps{g}") for g in range(2)]
                        st["ps"] = ps
                        views = [ps[0][:], ps[1][:]]
                    for f in range(16):
                        for g in range(2):
                            nc.tensor.matmul(
                                views[g],
                                C[:, (f * 2 + g) * 80:(f * 2 + g) * 80 + 80],
                                C[:, (32 + f * 2 + g) * 80:
                                  (32 + f * 2 + g) * 80 + 80],
                                start=(f == 0), stop=(f == 15),
                                skip_group_check=cfg["merge_so"])

                def s4():                   # PSUM -> SBUF -> DRAM
                    ps = st["ps"]
                    SO = pool.tile([80, 160], f32, tag="SO")
                    if cfg["merge_so"]:
                        nc.scalar.copy(SO[:], ps[:])
                    else:
                        nc.scalar.copy(SO[:, 0:80], ps[0][:])
                        nc.vector.tensor_scalar(SO[:, 80:160], ps[1][:], 1.0,
                                                None, OP.mult)
                    nc.sync.dma_start(sout, SO[:])

                return [s0, s1, s2, s3, s4]

            NST = 5

            def emit_block(n_bodies, pipelined):
                if not pipelined:
                    for u in range(n_bodies):
                        for s in stages(u):
                            s()
                    return
                live = {}
                for t in range(n_bodies + NST - 1):
                    u = t
                    if u < n_bodies:
                        live[u] = stages(u)
                    # deepest-first within a tick: the oldest body's tail
                    # retires ahead of newly enqueued work on each engine
                    for k in reversed(range(NST)):
                        v = t - k
                        if 0 <= v < n_bodies:
                            live[v][k]()
                    if t - NST + 1 in live:
                        del live[t - NST + 1]

            if loop:
                U = cfg["unroll"]
                assert reps % U == 0 and U % NCB == 0, (reps, U, NCB)
                with tc.For_i(0, reps // U, 1):
                    emit_block(U, cfg["stagger"])
            else:
                emit_block(reps, False)

    nc.compile()
    return nc


def _get_state():
    if "nc" not in _STATE:
        _STATE["nc"] = _build()
    return _STATE["nc"]


def _shard_inputs(x: np.ndarray):
    """x [512, 16, 32, 2] -> per-core {xx [128, 512]}.

    xx free layout: dd*256 + f*16 + b, partition p = tp*32 + agent where the
    64 local timesteps split as (f:16, tp:4).
    """
    in_maps = []
    for c in range(NCORES):
        xc = x[c * TLOC:(c + 1) * TLOC]            # [64, 16, 32, 2]
        arr = xc.reshape(16, 4, 16, 32, 2)         # (f, tp, b, a, d)
        arr = arr.transpose(4, 1, 3, 0, 2)         # (d, tp, a, f, b)
        arr = arr.reshape(2, 128, 256)             # p = tp*32+a, free = f*16+b
        xxc = np.concatenate([arr[0], arr[1]], axis=1)
        in_maps.append({"xx": np.ascontiguousarray(xxc)})
    return in_maps


def _gather(souts):
    """souts: list of 8 [80, 160] moment partials -> scalar loss (float32).

    sout row = pos0*8 + b', col (80*g + pos1*8 + b'') for batch b = 8*g+b',
    holding sum d0^m0 d1^m1 with m = POS2M[pos]. Host applies the
    Chebyshev transform A' then the reference normalization.
    """
    total = np.zeros((80, 160), dtype=np.float64)
    for s in souts:
        total += s.astype(np.float64)
    perm = np.array(POS2M)
    inv = np.empty(KMAX, dtype=np.int64)
    inv[perm] = np.arange(KMAX)
    Ap = _cheb_transform()                       # [k, m]
    Sm = np.empty((B, KMAX, KMAX), dtype=np.float64)
    for g in range(2):
        for bp in range(8):
            Mo = total[bp::8, 80 * g + bp:80 * (g + 1):8]   # [pos0, pos1]
            Mo = Mo[inv][:, inv]                             # [m0, m1]
            Sm[8 * g + bp] = Ap @ Mo @ Ap.T
    nf, nw, cd = _np_constants()
    coeffs = Sm.reshape(B, KN) / (NA * T) / nf[None, :].astype(np.float64)
    d = nw[None, :].astype(np.float64) * (coeffs - cd[None, :].astype(np.float64))
    loss = np.mean(d * d)
    return np.float32(loss)


def kernel(x: np.ndarray) -> np.ndarray:
    from concourse.bass_utils import run_bass_kernel_spmd

    nc = _get_state()
    in_maps = _shard_inputs(np.asarray(x, dtype=np.float32))
    res = run_bass_kernel_spmd(nc, in_maps, list(range(NCORES)))
    souts = [r["sout"] for r in res.results]
    return _gather(souts)
